# revision 30
# baseline (speedup 1.0000x reference)
"""AttentionDecoder Bass/Tile kernel for 8 Trainium2 NeuronCores.

Shapes (hardcoded): B=4, L=256, H=256, HEADS=4, D=64, BLOCKS=2.

Sharding: core c -> (batch b=c//2, query-half half=c%2). Each core computes
attention + FFN for its 128 query rows over all 256 keys (softmax row-local).
Block 2's K/V projections need block-1-updated seqs for all 256 rows -> one
AllGather within core pairs [[0,1],[2,3],[4,5],[6,7]] between blocks.

Time matrices are cast to bf16 and permuted host-side to [h, q, d, k].

Score matrix is built TRANSPOSED in PSUM (wT[k, q]) entirely on the tensor
engine: init with mask^T (DVE copy), accumulate K'^T x Q (K' has apK/Kb
folded), then per q-pair one matmul with the tmK slab as stationary
[128(2q x 64d), 128k] and two zero-padded Q columns as moving -> accumulates
sum_d tmK[q,k,d] Q[q,d] into wT columns. PE transpose back to [q, k],
softmax via ACT Exp(accum_out), unnormalized E used for A.V' matmuls and
the tmV einsum (DVE bcast-mult + bf16 tree-reduce); 1/sum folded into the
per-head epilogue scale.
"""
import os
import sys
import types
import numpy as np
import ml_dtypes

import concourse.bass as bass
import concourse.tile as tile
from concourse import bacc, mybir
from concourse.bass_utils import run_bass_kernel_spmd
from concourse.masks import make_identity

B, L, H, HEADS, BLOCKS = 4, 256, 256, 4, 2
D = H // HEADS            # 64
P = 128                   # partition dim / per-core query rows
N_CORES = 8
EPS = 1e-8
NEG = -4294967295.0
F32 = mybir.dt.float32
BF16 = mybir.dt.bfloat16
F8 = mybir.dt.float8e4
BF = ml_dtypes.bfloat16
F8NP = ml_dtypes.float8_e4m3

LAST_EXEC_TIME_NS = None


def _install_profile_hook():
    """Register the axon NTFF profile hook if absent (enables trace=True /
    BASS_TRACE=1 to return exec_time_ns). Best-effort."""
    try:
        try:
            from antenv.axon_hooks import get_axon_ntff_profile_hook  # noqa: F401
            return
        except ImportError:
            pass
        mod = types.ModuleType("antenv.axon_hooks")
        mod._hook = None
        mod.set_axon_ntff_profile_hook = lambda h: setattr(mod, "_hook", h)
        mod.get_axon_ntff_profile_hook = lambda: mod._hook
        sys.modules["antenv.axon_hooks"] = mod
        import antenv
        antenv.axon_hooks = mod
        from trn_agent_boot.trn_boot import _ntff_profile_via_ctypes
        so = "/opt/axon/libaxon_pjrt.so"
        if os.path.exists(so):
            mod.set_axon_ntff_profile_hook(_ntff_profile_via_ctypes(so))
        from concourse import bass_utils
        bass_utils.upload_artifacts = lambda tmpdir: "local://" + tmpdir
    except Exception:
        pass


def _bcast_mid(ap_2d, count):
    """[P, n] AP -> [P, count(bcast), n] AP (step-0 middle dim)."""
    return bass.AP(tensor=ap_2d.tensor, offset=ap_2d.offset,
                   ap=[ap_2d.ap[0], [0, count], ap_2d.ap[1]])


def _build_nc():
    nc = bacc.Bacc("TRN2", target_bir_lowering=False, debug=False,
                   num_devices=N_CORES)

    def din(name, shape, dt=F32):
        return nc.dram_tensor(name, list(shape), dt, kind="ExternalInput").ap()

    seqs_q0 = din("seqs_q0", [P, H])          # own query rows of seqs
    seqsT0 = din("seqsT0", [H, L])            # seqs[b].T for block-1 K/V
    qwT = din("qwT", [BLOCKS, H, H])          # (Qw*ln_attn).T * scale
    qbT = din("qbT", [BLOCKS, H, 1])          # Qb[:,None] * scale
    kwT = din("kwT", [BLOCKS, H, H])          # Kw.T
    apKTp = din("apKTp", [BLOCKS, H, L])      # apK[b].T + Kb[:,None]
    vwT = din("vwT", [BLOCKS, H, H])          # Vw.T
    apVp = din("apVp", [BLOCKS, L, H])        # apV[b] + Vb[None,:]
    w1T = din("w1T", [BLOCKS, H, H])          # (W1*ln_ffn).T
    b1T = din("b1T", [BLOCKS, H, 1])
    w2T = din("w2T", [BLOCKS, H, H])          # W2.T
    b2r = din("b2r", [BLOCKS, 1, H])
    lnl = din("lnl", [1, H])                  # ln_last row
    maskT = din("maskT", [L, P])              # additive mask transposed [k, q]
    maskN = din("maskN", [P, L])              # additive mask natural [q, k]
    notpad = din("notpad", [P, 1])
    tmk = din("tmk", [HEADS, P // 8, P, 4, L], F8)  # [h, jc, (e d), j, k] fp8
    tmv = din("tmv", [2, P, D, L], BF16)      # heads 0-1 [h, q, d, k] bf16
    tmv8 = din("tmv8", [2, P, D, L], F8)      # heads 2-3 [h, q, d, k] fp8

    y = nc.dram_tensor("y", [P, H], F32, kind="ExternalOutput").ap()

    kdebug = os.environ.get("KDEBUG") == "1"
    dbg_outs = {}

    from contextlib import ExitStack
    with tile.TileContext(nc) as tc, ExitStack() as ctx:
        const = ctx.enter_context(tc.tile_pool(name="const", bufs=1))
        work = ctx.enter_context(tc.tile_pool(name="work", bufs=2))
        tkp = ctx.enter_context(tc.tile_pool(name="tkp", bufs=3))
        tvp = ctx.enter_context(tc.tile_pool(name="tvp", bufs=3))
        dram = ctx.enter_context(tc.tile_pool(name="dram", bufs=1, space="DRAM"))
        psA = ctx.enter_context(tc.tile_pool(name="psA", bufs=1, space="PSUM"))
        psT = ctx.enter_context(tc.tile_pool(name="psT", bufs=2, space="PSUM"))
        psW = ctx.enter_context(tc.tile_pool(name="psW", bufs=2, space="PSUM"))
        psWw = ctx.enter_context(tc.tile_pool(name="psWw", bufs=2, space="PSUM"))
        psO = ctx.enter_context(tc.tile_pool(name="psO", bufs=1, space="PSUM"))

        ident = const.tile([P, P], F32)
        make_identity(nc, ident)
        for _n in range(int(os.environ.get("KNONCE", "0"))):
            nonce_sb = const.tile([P, 1], F32, name=f"nonce{_n}")
            nc.vector.memset(nonce_sb, float(_n))
        eps_sb = const.tile([P, 1], F32)
        nc.vector.memset(eps_sb, EPS)
        zero_sb = const.tile([P, 1], F32)
        nc.vector.memset(zero_sb, 0.0)
        zeros_bf = const.tile([P, L], BF16)
        nc.vector.memset(zeros_bf, 0.0)

        # ---- load constants ----
        def load3(name, src, i):  # [256, n] -> [128, 2, n]
            t = const.tile([P, 2, src.shape[-1]], F32, name=name)
            nc.sync.dma_start(out=t, in_=src[i].rearrange("(t p) n -> p t n", p=P))
            return t

        seqs_q = work.tile([P, H], F32, name="seqs_q")
        nc.sync.dma_start(out=seqs_q, in_=seqs_q0)
        seqsT = work.tile([P, 2, L], F32, name="seqsT")
        nc.sync.dma_start(out=seqsT, in_=seqsT0.rearrange("(t p) n -> p t n", p=P))
        maskN_sb = const.tile([P, L], F32)
        nc.sync.dma_start(out=maskN_sb, in_=maskN)
        qwT_sb = [load3(f"qwT{i}", qwT, i) for i in range(BLOCKS)]
        qbT_sb = [load3(f"qbT{i}", qbT, i) for i in range(BLOCKS)]
        kwT_sb = [load3(f"kwT{i}", kwT, i) for i in range(BLOCKS)]
        vwT_sb = [load3(f"vwT{i}", vwT, i) for i in range(BLOCKS)]
        apKT_sb = [load3(f"apKT{i}", apKTp, i) for i in range(BLOCKS)]
        apV_sb = [load3(f"apV{i}", apVp, i) for i in range(BLOCKS)]
        w1T_sb = [load3(f"w1T{i}", w1T, i) for i in range(BLOCKS)]
        w2T_sb = [load3(f"w2T{i}", w2T, i) for i in range(BLOCKS)]
        b1T_sb = [load3(f"b1T{i}", b1T, i) for i in range(BLOCKS)]

        b2r_sb = []
        for i in range(BLOCKS):
            t = const.tile([P, H], F32, name=f"b2r{i}")
            nc.sync.dma_start(out=t, in_=b2r[i].to_broadcast((P, H)))
            b2r_sb.append(t)
        lnl_sb = const.tile([P, H], F32)
        nc.sync.dma_start(out=lnl_sb, in_=lnl.to_broadcast((P, H)))
        np_sb = const.tile([P, 1], F32)
        nc.sync.dma_start(out=np_sb, in_=notpad)

        from concourse.bass import _add_dep_helper as _dep

        def rmsnorm_rows(x_sb, tag):
            """Returns rs [P,1] f32 = 1/sqrt(mean(x^2)+eps) for rows of x."""
            scr = work.tile([P, H], F32, name="scr_rms")
            ssq = work.tile([P, 1], F32, name=f"ssq_{tag}")
            a1 = nc.scalar.activation(out=scr, in_=x_sb,
                                 func=mybir.ActivationFunctionType.Square,
                                 bias=zero_sb, accum_out=ssq)
            sq = work.tile([P, 1], F32, name=f"sq_{tag}")
            a2 = nc.scalar.activation(out=sq, in_=ssq,
                                 func=mybir.ActivationFunctionType.Sqrt,
                                 scale=1.0 / H, bias=eps_sb)
            # accum_out (2nd output) dep is not tracked by Tile; force it
            _dep(a2.ins, a1.ins, sync=True, reason="sqrt reads Square accum_out")
            rs = work.tile([P, 1], F32, name=f"rs_{tag}")
            nc.vector.reciprocal(rs, sq)
            return rs

        def DBG(name, sbuf_ap):
            if not kdebug or name in dbg_outs:
                return
            shp = list(sbuf_ap.shape)
            dbg_outs[name] = nc.dram_tensor(
                f"dbg_{name}", shp, sbuf_ap.dtype, kind="ExternalOutput").ap()
            nc.sync.dma_start(out=dbg_outs[name], in_=sbuf_ap)

        def transpose128(src_ap, dst_ap):
            """PE-transpose a [128,128] SBUF slice into a [128,128] dst slice."""
            pt = psT.tile([P, P], F32, name="pt", tag="pt")
            nc.tensor.transpose(pt, src_ap, ident)
            nc.scalar.copy(dst_ap, pt)

        for i in range(BLOCKS):
            # ---- Q projection (from rms-normed own rows) ----
            rs = rmsnorm_rows(seqs_q, f"q{i}")
            x_sb = work.tile([P, H], F32, name="x_sb")
            nc.vector.tensor_scalar_mul(x_sb, seqs_q, rs)
            xT = work.tile([P, 2, P], F32, name="xT")
            for ct in range(2):
                transpose128(x_sb[:, ct * P:(ct + 1) * P], xT[:, ct, :])

            qt = work.tile([P, 2, P], F32, name="qt")  # Q^T [hd, q]
            for ht in range(2):
                pq = psA.tile([P, P], F32, name="pq", tag="psA_t")
                for ct in range(2):
                    nc.tensor.matmul(pq, qwT_sb[i][:, ct, ht * P:(ht + 1) * P],
                                     xT[:, ct, :], start=(ct == 0), stop=(ct == 1))
                nc.scalar.activation(out=qt[:, ht, :], in_=pq,
                                     func=mybir.ActivationFunctionType.Identity,
                                     bias=qbT_sb[i][:, ht, :])

            DBG("qt", qt)
            # zero-padded Q pair-columns per head: [128(2x64d), q] bf16
            # col 2j   : partitions [0:64)   = Q[:, 2j],  [64:128) = 0
            # col 2j+1 : partitions [0:64)   = 0,         [64:128) = Q[:, 2j+1]
            qz = work.tile([P, HEADS, P], BF16, name="qz")
            nc.vector.memset(qz, 0.0)
            for h in range(HEADS):
                ht, hp = h // 2, (h % 2) * D
                nc.scalar.copy(qz[0:D, h, 0::2], qt[hp:hp + D, ht, 0::2])
                nc.scalar.copy(qz[D:2 * D, h, 1::2], qt[hp:hp + D, ht, 1::2])

            DBG("qz", qz)
            # ---- K^T (+apK+Kb) and V (+apV+Vb) over full L ----
            ktp = work.tile([P, 2, L], F32, name="ktp")  # [hd, k]
            for ht in range(2):
                pk = psA.tile([P, L], F32, name="pk", tag="psA_t")
                for ct in range(2):
                    nc.tensor.matmul(pk, kwT_sb[i][:, ct, ht * P:(ht + 1) * P],
                                     seqsT[:, ct, :], start=(ct == 0), stop=(ct == 1))
                nc.vector.tensor_tensor(out=ktp[:, ht, :], in0=pk,
                                        in1=apKT_sb[i][:, ht, :],
                                        op=mybir.AluOpType.add)
            vp = work.tile([P, 2, H], F32, name="vp")  # [k, hd] in 2 k-tiles
            for kt in range(2):
                pv = psA.tile([P, H], F32, name="pv", tag="psA_t")
                for ct in range(2):
                    nc.tensor.matmul(pv, seqsT[:, ct, kt * P:(kt + 1) * P],
                                     vwT_sb[i][:, ct, :], start=(ct == 0), stop=(ct == 1))
                nc.vector.tensor_tensor(out=vp[:, kt, :], in0=pv,
                                        in1=apV_sb[i][:, kt, :],
                                        op=mybir.AluOpType.add)

            DBG("ktp", ktp)
            DBG("vp", vp)
            # ---- attention ----
            out_ps = psO.tile([P, H], F32, name="out_ps")
            otv = work.tile([P, H], F32, name="otv")   # tmV contribution
            rinv4 = work.tile([P, HEADS], F32, name="rinv4")
            for h in range(HEADS):
                ht, hp = h // 2, (h % 2) * D
                # wT[k, q] built in PSUM: mask^T + K'^T.T Q + tmK columns
                wTp = psW.tile([P, 2, P], F32, name="wTp")
                nc.tensor.matmul(wTp.rearrange("p a b -> p (a b)"),
                                 qz[:, h, :], zeros_bf,
                                 start=True, stop=False, skip_group_check=True)
                # tmK: stationary = tk slab [128(2q x 64d), 128k], moving =
                # 2 zero-padded Q cols -> wT[:, 2j:2j+2] += sum_d tmK*Q
                NJ = 4  # q-pairs per DMA tile
                for jc in range(P // (2 * NJ)):
                    tk = tkp.tile([P, NJ, 2, P], F8, name="tk")
                    nc.sync.dma_start(
                        out=tk,
                        in_=tmk[h, jc].rearrange("p j (t k) -> p j t k", k=P))
                    for j in range(NJ):
                        jj = jc * NJ + j
                        for kt in range(2):
                            nc.tensor.matmul(
                                wTp[:, kt, 2 * jj:2 * jj + 2],
                                tk[:, j, kt, :], qz[:, h, 2 * jj:2 * jj + 2],
                                start=False, stop=False,
                                skip_group_check=True)
                # K'^T x Q last: only this matmul waits on the AllGathered K
                for kt in range(2):
                    nc.tensor.matmul(wTp[:, kt, :],
                                     ktp[hp:hp + D, ht, kt * P:(kt + 1) * P],
                                     qt[hp:hp + D, ht, :],
                                     start=False, stop=True,
                                     skip_group_check=True)
                # transpose wT -> w [q, k]
                wts = work.tile([P, 2, P], F32, name="wts")
                nc.scalar.copy(wts[:, 0, :], wTp[:, 0, :])
                nc.scalar.copy(wts[:, 1, :], wTp[:, 1, :])
                DBG("wts", wts)
                w_ps = psWw.tile([P, L], F32, name="w_ps")
                for kt in range(2):
                    pt = psT.tile([P, P], F32, name="pt", tag="pt")
                    nc.tensor.transpose(pt, wts[:, kt, :], ident)
                    nc.scalar.copy(w_ps[:, kt * P:(kt + 1) * P], pt)
                # softmax over k (unnormalized E; 1/r folded into epilogue)
                s2 = work.tile([P, L], F32, name="s2")
                nc.vector.tensor_tensor(out=s2, in0=w_ps, in1=maskN_sb,
                                        op=mybir.AluOpType.add)
                m = work.tile([P, 1], F32, name="m")
                nc.vector.reduce_max(m, s2, axis=mybir.AxisListType.X)
                negm = work.tile([P, 1], F32, name="negm")
                nc.vector.tensor_scalar_mul(negm, m, -1.0)
                e_sb = work.tile([P, L], F32, name="e_sb")
                r_sb = work.tile([P, 1], F32, name="r_sb")
                a_exp = nc.scalar.activation(out=e_sb, in_=s2,
                                     func=mybir.ActivationFunctionType.Exp,
                                     bias=negm, accum_out=r_sb)
                DBG("e", e_sb)
                rec = nc.vector.reciprocal(rinv4[:, h:h + 1], r_sb)
                _dep(rec.ins, a_exp.ins, sync=True,
                     reason="reciprocal reads Exp accum_out")
                e_bf = work.tile([P, L], BF16, name="e_bf")
                ebf_cp = nc.scalar.copy(e_bf, e_sb)
                # E^T for E@V' matmul
                et = work.tile([P, 2, P], F32, name="et")
                for kt in range(2):
                    transpose128(e_sb[:, kt * P:(kt + 1) * P], et[:, kt, :])
                for kt in range(2):
                    nc.tensor.matmul(out_ps[:, h * D:(h + 1) * D], et[:, kt, :],
                                     vp[:, kt, h * D:(h + 1) * D],
                                     start=(kt == 0), stop=(kt == 1))
                # out_tmv[q, d] = sum_k E[q,k] * tmV[h,q,d,k]
                DC = 32
                eb_h = _bcast_mid(e_bf, DC)
                for dc in range(D // DC):
                    if h < 2:
                        tv = tvp.tile([P, DC, L], BF16, name="tv")
                        nc.sync.dma_start(out=tv,
                                          in_=tmv[h, :, dc * DC:(dc + 1) * DC, :])
                    else:
                        tv = tvp.tile([P, DC, L], F8, name="tv8")
                        nc.sync.dma_start(out=tv,
                                          in_=tmv8[h - 2, :, dc * DC:(dc + 1) * DC, :])
                    prodv = tvp.tile([P, DC, L], BF16, name="prodv", bufs=2)
                    pm = nc.vector.tensor_tensor(out=prodv, in0=tv, in1=eb_h,
                                                 op=mybir.AluOpType.mult)
                    _dep(pm.ins, ebf_cp.ins, sync=True,
                         reason="prodv mult reads e_bf broadcast AP")
                    # bf16 tree-halving over k (3 levels), then 1x reduce
                    for lv in (L // 2, L // 4, L // 8):
                        nc.vector.tensor_tensor(
                            out=prodv[:, :, 0:lv], in0=prodv[:, :, 0:lv],
                            in1=prodv[:, :, lv:2 * lv], op=mybir.AluOpType.add)
                    d_idx = h * D + dc * DC
                    nc.vector.reduce_sum(otv[:, d_idx:d_idx + DC],
                                         prodv[:, :, 0:L // 8],
                                         axis=mybir.AxisListType.X)

            DBG("otv", otv)
            # ---- epilogue: normalize heads, residual ----
            sm = work.tile([P, H], F32, name="sm")
            nc.vector.tensor_tensor(out=sm, in0=out_ps, in1=otv,
                                    op=mybir.AluOpType.add)
            sm2 = work.tile([P, H], F32, name="sm2")
            for h in range(HEADS):
                nc.vector.tensor_scalar_mul(sm2[:, h * D:(h + 1) * D],
                                            sm[:, h * D:(h + 1) * D],
                                            rinv4[:, h:h + 1])
            DBG("sm", sm)
            DBG("sm2", sm2)
            seqs_mid = work.tile([P, H], F32, name="seqs_mid")
            nc.vector.tensor_tensor(out=seqs_mid, in0=sm2, in1=seqs_q,
                                    op=mybir.AluOpType.add)

            DBG("seqs_mid", seqs_mid)
            # ---- FFN ----
            rs2 = rmsnorm_rows(seqs_mid, f"f{i}")
            h2 = work.tile([P, H], F32, name="h2")
            nc.vector.tensor_scalar_mul(h2, seqs_mid, rs2)
            h2T = work.tile([P, 2, P], F32, name="h2T")
            for ct in range(2):
                transpose128(h2[:, ct * P:(ct + 1) * P], h2T[:, ct, :])
            fT = work.tile([P, 2, P], F32, name="fT")
            for ht in range(2):
                pf = psA.tile([P, P], F32, name="pf", tag="psA_t")
                for ct in range(2):
                    nc.tensor.matmul(pf, w1T_sb[i][:, ct, ht * P:(ht + 1) * P],
                                     h2T[:, ct, :], start=(ct == 0), stop=(ct == 1))
                nc.scalar.activation(out=fT[:, ht, :], in_=pf,
                                     func=mybir.ActivationFunctionType.Relu,
                                     bias=b1T_sb[i][:, ht, :])
            po2 = psA.tile([P, H], F32, name="po2", tag="psA_t")
            for ct in range(2):
                nc.tensor.matmul(po2, fT[:, ct, :], w2T_sb[i][:, ct, :],
                                 start=(ct == 0), stop=(ct == 1))
            f1 = work.tile([P, H], F32, name="f1")
            nc.vector.tensor_tensor(out=f1, in0=po2, in1=b2r_sb[i],
                                    op=mybir.AluOpType.add)
            f2 = work.tile([P, H], F32, name="f2")
            nc.vector.tensor_tensor(out=f2, in0=f1, in1=seqs_mid,
                                    op=mybir.AluOpType.add)
            seqs_q = work.tile([P, H], F32, name="seqs_q")
            nc.vector.tensor_scalar_mul(seqs_q, f2, np_sb)

            DBG(f"seqs_b{i}", seqs_q)
            # ---- exchange updated seqs within the core pair ----
            if i == 0:
                from concourse.bass import _add_dep_helper
                in_b = dram.tile([P, H], F32, name="in_b")
                out_b = dram.tile([L, H], F32, name="out_b")
                d_in = nc.sync.dma_start(out=in_b, in_=seqs_q)
                ag = nc.gpsimd.collective_compute(
                    "AllGather", mybir.AluOpType.bypass,
                    replica_groups=[[0, 1], [2, 3], [4, 5], [6, 7]],
                    ins=[in_b.opt()], outs=[out_b.opt()])
                _add_dep_helper(ag.ins, d_in.ins, sync=True,
                                reason="AllGather reads in_b after bounce write")
                seqs_full = work.tile([P, 2, H], F32, name="seqs_full")
                d_out = nc.sync.dma_start(out=seqs_full,
                                          in_=out_b.rearrange("(t p) n -> p t n", p=P))
                _add_dep_helper(d_out.ins, ag.ins, sync=True,
                                reason="seqs_full read waits on AllGather")
                DBG("seqs_full", seqs_full)
                seqsT = work.tile([P, 2, L], F32, name="seqsT")
                for lt in range(2):
                    for ct in range(2):
                        transpose128(seqs_full[:, lt, ct * P:(ct + 1) * P],
                                     seqsT[:, ct, lt * P:(lt + 1) * P])

        # ---- final rmsnorm(seqs, ln_last) ----
        rs3 = rmsnorm_rows(seqs_q, "last")
        fin1 = work.tile([P, H], F32, name="fin1")
        nc.vector.tensor_scalar_mul(fin1, seqs_q, rs3)
        fin2 = work.tile([P, H], F32, name="fin2")
        nc.vector.tensor_tensor(out=fin2, in0=fin1, in1=lnl_sb,
                                op=mybir.AluOpType.mult)
        nc.sync.dma_start(out=y, in_=fin2)

    nc.compile()
    return nc


_NC_CACHE = None


def _get_nc():
    global _NC_CACHE
    if _NC_CACHE is None:
        _install_profile_hook()
        _NC_CACHE = _build_nc()
    return _NC_CACHE


def _host_prep(seqs, attention_mask, timeline_mask, time_matrix_K, time_matrix_V,
               abs_pos_K, abs_pos_V, ln_attn, Qw, Qb, Kw, Kb, Vw, Vb,
               ln_ffn, W1, b1, W2, b2, ln_last):
    scale = np.float32(1.0 / np.sqrt(np.float32(D)))
    f = np.float32
    seqs = np.asarray(seqs, f)
    am = np.asarray(attention_mask, bool)
    tm = np.asarray(timeline_mask, bool)

    qwT = np.stack([(Qw[i] * ln_attn[i][None, :]).T * scale for i in range(BLOCKS)])
    qbT = np.stack([(Qb[i] * scale)[:, None] for i in range(BLOCKS)])
    kwT = np.stack([Kw[i].T for i in range(BLOCKS)])
    vwT = np.stack([Vw[i].T for i in range(BLOCKS)])
    w1T = np.stack([(W1[i] * ln_ffn[i][None, :]).T for i in range(BLOCKS)])
    b1T = np.stack([b1[i][:, None] for i in range(BLOCKS)])
    w2T = np.stack([W2[i].T for i in range(BLOCKS)])
    b2r = np.stack([b2[i][None, :] for i in range(BLOCKS)])
    lnl = np.asarray(ln_last, f)[None, :]

    shared = dict(
        qwT=np.ascontiguousarray(qwT, f), qbT=np.ascontiguousarray(qbT, f),
        kwT=np.ascontiguousarray(kwT, f), vwT=np.ascontiguousarray(vwT, f),
        w1T=np.ascontiguousarray(w1T, f), b1T=np.ascontiguousarray(b1T, f),
        w2T=np.ascontiguousarray(w2T, f), b2r=np.ascontiguousarray(b2r, f),
        lnl=np.ascontiguousarray(lnl, f),
    )

    in_maps = []
    for c in range(N_CORES):
        b, half = c // 2, c % 2
        qsl = slice(half * P, (half + 1) * P)
        apKTp = np.stack([abs_pos_K[b].T + Kb[i][:, None] for i in range(BLOCKS)])
        apVp = np.stack([abs_pos_V[b] + Vb[i][None, :] for i in range(BLOCKS)])
        maskb = tm[b][qsl, None] | am[qsl, :]
        maskadd = np.where(maskb, f(NEG), f(0.0))
        notpad = (1.0 - tm[b][qsl].astype(f))[:, None]
        tmk = time_matrix_K[b, qsl].reshape(P, L, HEADS, D).transpose(2, 0, 3, 1)
        # -> [h, jc=16, j=4, e=2, d, k] -> [h, jc, (e d), j, k]
        tmk = tmk.reshape(HEADS, 16, 4, 2, D, L).transpose(0, 1, 3, 4, 2, 5)
        tmk = tmk.reshape(HEADS, 16, P, 4, L)
        tmv = time_matrix_V[b, qsl].reshape(P, L, HEADS, D).transpose(2, 0, 3, 1)
        in_maps.append(dict(
            shared,
            seqs_q0=np.ascontiguousarray(seqs[b, qsl], f),
            seqsT0=np.ascontiguousarray(seqs[b].T, f),
            apKTp=np.ascontiguousarray(apKTp, f),
            apVp=np.ascontiguousarray(apVp, f),
            maskT=np.ascontiguousarray(maskadd.T, f),
            maskN=np.ascontiguousarray(maskadd, f),
            notpad=np.ascontiguousarray(notpad, f),
            tmk=np.ascontiguousarray(tmk).astype(F8NP),
            tmv=np.ascontiguousarray(tmv[:2]).astype(BF),
            tmv8=np.ascontiguousarray(tmv[2:]).astype(F8NP),
        ))
    return in_maps


def kernel(**inputs):
    global LAST_EXEC_TIME_NS
    nc = _get_nc()
    in_maps = _host_prep(**inputs)
    res = run_bass_kernel_spmd(nc, in_maps, core_ids=list(range(N_CORES)))
    LAST_EXEC_TIME_NS = res.exec_time_ns
    out = np.empty((B, L, H), np.float32)
    for c in range(N_CORES):
        b, half = c // 2, c % 2
        out[b, half * P:(half + 1) * P] = res.results[c]["y"]
    return out
